# revision 28
# baseline (speedup 1.0000x reference)
import sys
sys.path.insert(0, "/opt/trn_rl_repo")

import numpy as np
import ml_dtypes
from contextlib import ExitStack

import concourse.bass as bass
import concourse.mybir as mybir
import concourse.tile as tile
from concourse import bacc
from concourse.bass_utils import run_bass_kernel_spmd

# ---- problem constants (hardcoded per spec) ----
H, W = 200, 100
NQ, NV, EMB, HEADS, NPT, DH = 2000, 20000, 256, 8, 4, 32
N_CORES = 8
YP = 203                    # padded rows y' = y+1, y' in [0, 202]
NCELL = YP * W              # 20300
NCELL_PAD = 20352           # 159 * 128
NTILE_C = NCELL_PAD // 128  # 159
VEXT = 20452                # 100 zero cols + 20000 real + 352 zero
VEXTA = 20480               # vt allocation stride (ldweights needs alignment)
QT = 2048                   # padded queries (16 tiles of 128)
NQT = 16
F32 = mybir.dt.float32
BF16 = mybir.dt.bfloat16
FP8 = mybir.dt.float8e4
I16 = mybir.dt.int16
WV_SCALE = 32.0             # W_val prescale for fp8; undone via softmax recip

_CACHE = {}


def build_kernel(debug=False):
    nc = bacc.Bacc("TRN2", target_bir_lowering=False, debug=False,
                   num_devices=N_CORES, num_swdge_queues=4)
    A = mybir.AluOpType
    ACT = mybir.ActivationFunctionType
    dt = nc.dram_tensor
    q_in = dt("q", [NQ, EMB], F32, kind="ExternalInput")
    v_in = dt("v", [NV, EMB], F32, kind="ExternalInput")
    ref_in = dt("ref", [NQ, 2], F32, kind="ExternalInput")
    wv_in = dt("wv", [128, 2, EMB], FP8, kind="ExternalInput")
    woa_in = dt("woa", [2, 128, 96], BF16, kind="ExternalInput")
    boa_in = dt("boa", [96], F32, kind="ExternalInput")
    wout_in = dt("wout", [2, 128, 256], BF16, kind="ExternalInput")
    bout_in = dt("bout", [256], F32, kind="ExternalInput")
    e2s_in = dt("e2s", [128, 64 * 128], BF16, kind="ExternalInput")
    r2_in = dt("r2", [128, 32], BF16, kind="ExternalInput")
    idf_in = dt("idf", [128, 128], F32, kind="ExternalInput")
    idb_in = dt("idb", [128, 128], BF16, kind="ExternalInput")
    outT = dt("outT", [2, 128, NQ], F32, kind="ExternalOutput")
    if debug:
        dbg_oa = dt("dbg_oa", [128, NQT * 96], F32, kind="ExternalOutput")
        dbg_wc = dt("dbg_wc", [128, NQT * 128], F32, kind="ExternalOutput")
        dbg_idx = dt("dbg_idx", [64, QT], I16, kind="ExternalOutput")
        dbg_ct = dt("dbg_ct", [2, 128, QT], F32, kind="ExternalOutput")
        dbg_vp2 = dt("dbg_vp2", [NCELL_PAD * HEADS * 64], BF16,
                     kind="ExternalOutput")

    with tile.TileContext(nc) as tc, ExitStack() as octx:
        const = octx.enter_context(tc.tile_pool(name="const", bufs=1))
        dram = octx.enter_context(tc.tile_pool(name="dram", bufs=1,
                                               space="DRAM"))
        vp2t = dram.tile([NCELL_PAD * HEADS * 64], BF16, tag="vp2")
        idxd = dram.tile([64 * QT], I16, tag="idxd")

        idf = const.tile([128, 128], F32, tag="idf")
        nc.sync.dma_start(out=idf, in_=idf_in[:, :])
        idb = const.tile([128, 128], BF16, tag="idb")
        nc.sync.dma_start(out=idb, in_=idb_in[:, :])
        wv8 = const.tile([128, 2, EMB], FP8, tag="wv8")
        nc.sync.dma_start(out=wv8, in_=wv_in[:, :, :])
        woa = [const.tile([128, 96], BF16, tag=f"woa{ch}", name=f"woa{ch}") for ch in range(2)]
        wout = [const.tile([128, 256], BF16, tag=f"wo{ch}", name=f"wo{ch}") for ch in range(2)]
        for ch in range(2):
            nc.sync.dma_start(out=woa[ch], in_=woa_in[ch])
            nc.sync.dma_start(out=wout[ch], in_=wout_in[ch])
        bias_oa = const.tile([128, 96], F32, tag="boa")
        nc.gpsimd.dma_start(out=bias_oa, in_=bass.AP(
            tensor=boa_in, offset=0, ap=[[0, 128], [1, 96]]))
        bout_sb = const.tile([128, 2], F32, tag="bout")
        nc.gpsimd.dma_start(out=bout_sb, in_=bass.AP(
            tensor=bout_in, offset=0, ap=[[1, 128], [128, 2]]))

        persist = octx.enter_context(tc.tile_pool(name="persist", bufs=1))
        qTf = [persist.tile([128, QT], F32, tag=f"qTf{c}", name=f"qTf{c}") for c in range(2)]
        qTb = [persist.tile([128, QT], BF16, tag=f"qTb{c}", name=f"qTb{c}") for c in range(2)]
        oa_sb = persist.tile([128, NQT, 96], F32, tag="oa")
        wcb = persist.tile([128, NQT, 32, 4], BF16, tag="wcb")
        ctt = [persist.tile([128, QT], BF16, tag=f"ct{c}", name=f"ct{c}") for c in range(2)]

        # ---------- stage 0: transpose query -> qT ----------
        with tc.tile_pool(name="s0", bufs=3) as s0, \
             tc.tile_pool(name="s0p", bufs=2, space="PSUM") as s0p:
            for ch in range(2):
                for g in range(4):
                    ps = s0p.tile([128, 512], F32, tag="qt_ps")
                    for k in range(4):
                        t = g * 4 + k
                        qt = s0.tile([128, 256], F32, tag="q_ld")
                        if t == NQT - 1:
                            nc.vector.memset(qt, 0.0)
                            nc.scalar.dma_start(out=qt[:NQ - 1920, :],
                                                in_=q_in[1920:NQ, :])
                        else:
                            nc.scalar.dma_start(
                                out=qt, in_=q_in[t * 128:(t + 1) * 128, :])
                        nc.tensor.transpose(ps[:, k * 128:(k + 1) * 128],
                                            qt[:, ch * 128:(ch + 1) * 128],
                                            idf)
                    nc.scalar.activation(qTf[ch][:, g * 512:(g + 1) * 512],
                                         ps, ACT.Copy)
            for ch in range(2):
                nc.vector.tensor_copy(qTb[ch], qTf[ch])

        # ---------- stage 3: off/attn projections ----------
        with tc.tile_pool(name="s3p", bufs=2, space="PSUM") as s3p:
            for t in range(NQT):
                ps = s3p.tile([128, 96], F32, tag="oa_ps")
                for ch in range(2):
                    nc.tensor.matmul(ps, qTb[ch][:, t * 128:(t + 1) * 128],
                                     woa[ch], start=(ch == 0), stop=(ch == 1))
                nc.vector.tensor_add(oa_sb[:, t, :], ps, bias_oa)
            if debug:
                nc.sync.dma_start(out=dbg_oa[:, :],
                                  in_=oa_sb.rearrange("p t c -> p (t c)"))

        # ---------- stage 4: coords, weights, indices ----------
        with tc.tile_pool(name="s4", bufs=1) as s4, \
             tc.tile_pool(name="s4p", bufs=2, space="PSUM") as s4p:
            shp = [128, NQT, 32]
            ref_sb = s4.tile([128, NQT, 2], F32, tag="ref")
            nc.vector.memset(ref_sb, 0.0)
            nc.scalar.dma_start(
                out=ref_sb[:, 0:15, :],
                in_=ref_in[0:1920, :].rearrange("(t p) c -> p t c", p=128))
            nc.scalar.dma_start(out=ref_sb[:NQ - 1920, 15, :],
                                in_=ref_in[1920:NQ, :])
            oav = oa_sb.rearrange("p t (c two) -> p t c two", two=2)
            ox = oav[:, :, 0:32, 0]
            oy = oav[:, :, 0:32, 1]
            awl = oa_sb[:, :, 64:96]

            awe = s4.tile(shp, F32, tag="awe")
            nc.scalar.activation(awe, awl, ACT.Exp)
            s1t = s4.tile([128, NQT, 16], F32, tag="s1t")
            av = awe.rearrange("p t (c two) -> p t c two", two=2)
            nc.vector.tensor_add(s1t, av[:, :, :, 0], av[:, :, :, 1])
            s2t = s4.tile([128, NQT, 8], F32, tag="s2t")
            sv = s1t.rearrange("p t (c two) -> p t c two", two=2)
            nc.vector.tensor_add(s2t, sv[:, :, :, 0], sv[:, :, :, 1])
            # fold the fp8 W_val prescale out of the softmax normalization
            nc.vector.tensor_scalar(out=s2t, in0=s2t, scalar1=WV_SCALE,
                                    scalar2=None, op0=A.mult)
            rec = s4.tile([128, NQT, 8], F32, tag="rec")
            nc.vector.reciprocal(rec, s2t)
            awn = s4.tile(shp, F32, tag="awn")
            nc.vector.tensor_mul(
                awn.rearrange("p t (c f) -> p t c f", f=4),
                awe.rearrange("p t (c f) -> p t c f", f=4),
                rec[:, :, :, None].broadcast_to([128, NQT, 8, 4]))

            refw = s4.tile([128, NQT, 2], F32, tag="refw")
            nc.vector.tensor_scalar(out=refw[:, :, 0:1],
                                    in0=ref_sb[:, :, 0:1],
                                    scalar1=float(W), scalar2=0.5,
                                    op0=A.mult, op1=A.add)
            nc.vector.tensor_scalar(out=refw[:, :, 1:2],
                                    in0=ref_sb[:, :, 1:2],
                                    scalar1=float(H), scalar2=0.5,
                                    op0=A.mult, op1=A.add)
            px = s4.tile(shp, F32, tag="px")
            nc.vector.tensor_add(px, ox, refw[:, :, 0:1].broadcast_to(shp))
            py = s4.tile(shp, F32, tag="py")
            nc.vector.tensor_add(py, oy, refw[:, :, 1:2].broadcast_to(shp))
            nc.vector.tensor_scalar(out=px, in0=px, scalar1=0.0,
                                    scalar2=float(W + 1),
                                    op0=A.max, op1=A.min)
            nc.vector.tensor_scalar(out=py, in0=py, scalar1=0.0,
                                    scalar2=float(YP - 2),
                                    op0=A.max, op1=A.min)
            M23 = float(1 << 23)
            x0 = s4.tile(shp, F32, tag="x0")
            nc.vector.tensor_scalar(out=x0, in0=px, scalar1=M23 - 0.5,
                                    scalar2=M23, op0=A.add, op1=A.subtract)
            y0 = s4.tile(shp, F32, tag="y0")
            nc.vector.tensor_scalar(out=y0, in0=py, scalar1=M23 - 0.5,
                                    scalar2=M23, op0=A.add, op1=A.subtract)
            fx = s4.tile(shp, F32, tag="fx")
            nc.vector.tensor_sub(fx, px, x0)
            fy = s4.tile(shp, F32, tag="fy")
            nc.vector.tensor_sub(fy, py, y0)

            idxf = s4.tile([128, NQT, 64], F32, tag="idxf")
            idv = idxf.rearrange("p t (c two) -> p t c two", two=2)
            cellf = s4.tile(shp, F32, tag="cellf")
            nc.vector.scalar_tensor_tensor(out=cellf, in0=y0,
                                           scalar=float(W), in1=x0,
                                           op0=A.mult, op1=A.add)
            nc.vector.tensor_scalar(out=idv[:, :, :, 0], in0=cellf,
                                    scalar1=1.0, scalar2=0.0,
                                    op0=A.subtract, op1=A.max)
            nc.vector.tensor_scalar(out=idv[:, :, :, 0], in0=idv[:, :, :, 0],
                                    scalar1=float(NCELL - 1), scalar2=None,
                                    op0=A.min)
            nc.vector.tensor_scalar(out=idv[:, :, :, 1], in0=idv[:, :, :, 0],
                                    scalar1=1.0, scalar2=float(NCELL),
                                    op0=A.add, op1=A.min)

            ga1 = s4.tile(shp, F32, tag="ga1")
            nc.vector.tensor_scalar(out=ga1, in0=x0, scalar1=0.5,
                                    scalar2=None, op0=A.is_ge)
            ga2 = s4.tile(shp, F32, tag="ga2")
            nc.vector.tensor_scalar(out=ga2, in0=x0, scalar1=float(W) + 0.5,
                                    scalar2=None, op0=A.is_le)
            gb = s4.tile(shp, F32, tag="gb")
            nc.vector.tensor_scalar(out=gb, in0=x0, scalar1=float(W) - 0.5,
                                    scalar2=None, op0=A.is_le)
            fx1 = s4.tile(shp, F32, tag="fx1")
            nc.vector.tensor_scalar(out=fx1, in0=fx, scalar1=-1.0,
                                    scalar2=1.0, op0=A.mult, op1=A.add)
            fy1 = s4.tile(shp, F32, tag="fy1")
            nc.vector.tensor_scalar(out=fy1, in0=fy, scalar1=-1.0,
                                    scalar2=1.0, op0=A.mult, op1=A.add)
            aa = s4.tile(shp, F32, tag="aa")
            nc.vector.tensor_mul(aa, fx1, ga1)
            nc.vector.tensor_mul(aa, aa, ga2)
            nc.vector.tensor_mul(aa, aa, awn)
            bb = s4.tile(shp, F32, tag="bb")
            nc.vector.tensor_mul(bb, fx, gb)
            nc.vector.tensor_mul(bb, bb, awn)

            wc = s4.tile([128, NQT, 128], F32, tag="wc")
            wcv = wc.rearrange("p t (c j) -> p t c j", j=4)
            nc.vector.tensor_mul(wcv[:, :, :, 0], aa, fy1)
            nc.vector.tensor_mul(wcv[:, :, :, 1], aa, fy)
            nc.vector.tensor_mul(wcv[:, :, :, 2], bb, fy1)
            nc.vector.tensor_mul(wcv[:, :, :, 3], bb, fy)
            if debug:
                nc.sync.dma_start(out=dbg_wc[:, :],
                                  in_=wc.rearrange("p t c -> p (t c)"))

            nc.scalar.activation(
                wcb, wc.rearrange("p t (c j) -> p t c j", j=4), ACT.Copy)
            idxT = s4.tile([64, QT], F32, tag="idxT")
            for t in range(NQT):
                ps2 = s4p.tile([128, 128], F32, tag="tr2_ps")
                nc.tensor.transpose(ps2[:64, :], idxf[:, t, :], idf)
                dstv = idxT.rearrange("p (v tt u) -> p v tt u",
                                      v=16, tt=16)[:, :, t, :]
                srcv = ps2[:64, :].rearrange("p (u v) -> p v u", u=8, v=16)
                nc.vector.tensor_copy(dstv, srcv)
            idx16 = s4.tile([64, QT], I16, tag="idx16")
            nc.vector.tensor_copy(idx16, idxT)
            nc.scalar.dma_start(
                out=idxd[:].rearrange("(p f) -> p f", p=64), in_=idx16)
            if debug:
                nc.sync.dma_start(out=dbg_idx[:, :], in_=idx16)

        # ---------- stage 1+2: value transpose, fp8 projection, vp2 --------
        # vp2 is four head-pair tables: [pair][cell][2h][2s][32] bf16, so a
        # single 512B gather packet covers cells (c, c+1) for one head pair.
        with tc.tile_pool(name="vtp", bufs=1) as vtp:
            vt = vtp.tile([128, 2, VEXTA], FP8, tag="vt")
            for ch in range(2):
                nc.vector.memset(vt[:, ch, 0:100], 0.0)
                nc.vector.memset(vt[:, ch, 20100:VEXT], 0.0)
            with tc.tile_pool(name="s1f", bufs=5) as s1f, \
                 tc.tile_pool(name="s1", bufs=4) as s1, \
                 tc.tile_pool(name="s1p", bufs=2, space="PSUM") as s1p:
                NT = 157
                for g in range(40):
                    tiles = [t for t in range(g * 4, min(g * 4 + 4, NT))]
                    vbf = []
                    for t in tiles:
                        vb32 = s1f.tile([128, 256], F32, tag="v_ld32")
                        vb = s1.tile([128, 256], BF16, tag="v_ld")
                        rows = min(128, NV - t * 128)
                        if rows < 128:
                            nc.vector.memset(vb, 0.0)
                        nc.sync.dma_start(
                            out=vb32[:rows, :],
                            in_=v_in[t * 128:t * 128 + rows, :])
                        if t % 2 == 0:
                            nc.vector.tensor_copy(vb[:rows, :], vb32[:rows, :])
                        else:
                            nc.scalar.activation(vb[:rows, :], vb32[:rows, :],
                                                 ACT.Copy)
                        vbf.append(vb)
                    for ch in range(2):
                        ps = s1p.tile([128, 512], BF16, tag="vt_ps")
                        for k, vb in enumerate(vbf):
                            nc.tensor.transpose(
                                ps[:, k * 128:(k + 1) * 128],
                                vb[:, ch * 128:(ch + 1) * 128], idb)
                        cols = len(vbf) * 128
                        nc.scalar.activation(
                            vt[:, ch, 100 + g * 512:100 + g * 512 + cols],
                            ps[:, :cols], ACT.Copy)

            with tc.tile_pool(name="s2", bufs=3) as s2, \
                 tc.tile_pool(name="s2p", bufs=2, space="PSUM") as s2p:
                DR = mybir.MatmulPerfMode.DoubleRow
                for ci in range(NTILE_C):
                    ps = s2p.tile([128, 512], F32, tag="pj_ps")
                    c0 = ci * 128
                    nc.tensor.matmul(ps[:, 0:256], vt[:, :, c0:c0 + 128],
                                     wv8, start=True, stop=True,
                                     perf_mode=DR)
                    nc.tensor.matmul(ps[:, 256:512],
                                     vt[:, :, c0 + 100:c0 + 228],
                                     wv8, start=True, stop=True,
                                     perf_mode=DR)
                    mx = s2.tile([128, 512], BF16, tag="mx")
                    src = ps.rearrange("p (s h d) -> p h s d", s=2, h=8)
                    dst = mx.rearrange("p (h s d) -> p h s d", h=8, s=2)
                    if ci % 2 == 0:
                        nc.vector.tensor_copy(dst, src)
                    else:
                        nc.scalar.activation(dst, src, ACT.Copy)
                    nc.sync.dma_start(
                        out=bass.AP(
                            tensor=vp2t[:].tensor,
                            offset=vp2t[:].offset + ci * 16384,
                            ap=[[128, 128], [NCELL_PAD * 128, 4], [1, 128]]),
                        in_=mx.rearrange("p (pr e) -> p pr e", pr=4))
        if debug:
            nc.sync.dma_start(out=dbg_vp2[:], in_=vp2t[:])

        # ---------- stage 5: gather (q-major) + weighted combine on DVE ----
        # Non-transpose gathers are multi-queue safe (no xbar state); output
        # lands query-major: g[p, t, e] = packet for query q = t*128+p.
        vp2full = vp2t[:]
        with tc.tile_pool(name="s5i", bufs=1) as s5i, \
             tc.tile_pool(name="s5", bufs=8) as s5, \
             tc.tile_pool(name="s5b", bufs=4) as s5b, \
             tc.tile_pool(name="s5a", bufs=2) as s5a:
            # all 64 wrapped idx rows in one load (keeps GPSIMD free of
            # small DMAs): iball[p, row, c] = idxd[row*QT + (p%16)*128 + c]
            iball = s5i.tile([128, 64, 128], I16, tag="iball")
            for rep in range(8):
                nc.scalar.dma_start(
                    out=iball[16 * rep:16 * rep + 16, :, :],
                    in_=bass.AP(
                        tensor=idxd[:].tensor, offset=idxd[:].offset,
                        ap=[[128, 16], [QT, 64], [1, 128]]))
            ctq = s5i.tile([128, NQT, HEADS, 32], BF16, tag="ctq")
            for h in range(HEADS):
                acc = s5a.tile([128, NQT, 32], F32, tag="acc")
                for p in range(NPT):
                    hp = h * 4 + p
                    g = s5.tile([128, NQT, 256], BF16, tag="g")
                    nc.gpsimd.dma_gather(
                        g,
                        bass.AP(tensor=vp2full.tensor,
                                offset=(vp2full.offset
                                        + (h // 2) * NCELL_PAD * 128
                                        + (h % 2) * 64),
                                ap=[[128, NCELL_PAD - 2], [1, 256]]),
                        iball[:, hp * 2, :], QT, QT, 256, elem_step=128,
                        transpose=False, single_packet=False,
                        queue_num=hp % 4)
                    # packet = [cell c: pair(2h x 2s x 32) | cell c+1: ...];
                    # useful quarters: x (cell) in {0,1}, head-slot h%2,
                    # rows r in {0,1} -> corner weight j = x*2 + r
                    gv = g.rearrange("p t (x q d) -> p t x q d",
                                     x=2, q=4)[:, :, :, 0:2, :]
                    m = s5b.tile([128, NQT, 2, 2, 32], BF16, tag="m")
                    w5 = wcb[:, :, hp, :].rearrange(
                        "p t (x r) -> p t x r", x=2)
                    nc.vector.tensor_mul(
                        m, gv,
                        w5[:, :, :, :, None].broadcast_to(
                            [128, NQT, 2, 2, 32]))
                    t2 = s5b.tile([128, NQT, 2, 32], BF16, tag="t2")
                    nc.vector.tensor_add(t2, m[:, :, :, 0, :],
                                         m[:, :, :, 1, :])
                    if p == 0:
                        nc.vector.tensor_add(acc, t2[:, :, 0, :],
                                             t2[:, :, 1, :])
                    else:
                        tmp = s5b.tile([128, NQT, 32], BF16, tag="tmp")
                        nc.vector.tensor_add(tmp, t2[:, :, 0, :],
                                             t2[:, :, 1, :])
                        nc.vector.tensor_add(acc, acc, tmp)
                nc.scalar.activation(ctq[:, :, h, :], acc, ACT.Copy)
                if h % 4 == 3:
                    # heads (h-3..h) complete -> transpose this emb half now
                    # so ch-0's transposes hide under heads 4-7's gathers
                    ch = h // 4
                    with tc.tile_pool(name=f"s5p{ch}", bufs=2,
                                      space="PSUM") as s5p:
                        for tg in range(4):
                            ps = s5p.tile([128, 512], BF16, tag="ct_ps")
                            for k in range(4):
                                t = tg * 4 + k
                                src = ctq.rearrange("p t h d -> p t (h d)")[
                                    :, t, ch * 128:(ch + 1) * 128]
                                nc.tensor.transpose(
                                    ps[:, k * 128:(k + 1) * 128], src, idb)
                            dst = ctt[ch][:, tg * 512:(tg + 1) * 512]
                            if tg % 2 == 0:
                                nc.vector.tensor_copy(dst, ps)
                            else:
                                nc.scalar.activation(dst, ps, ACT.Copy)

        # ---------- stage 6: output projection + identity ----------
        with tc.tile_pool(name="s6", bufs=2) as s6, \
             tc.tile_pool(name="s6p", bufs=2, space="PSUM") as s6p:
            for oh in range(2):
                ps = s6p.tile([128, QT], F32, tag="out_ps")
                for qc in range(4):
                    for ch in range(2):
                        nc.tensor.matmul(
                            ps[:, qc * 512:(qc + 1) * 512],
                            wout[ch][:, oh * 128:(oh + 1) * 128],
                            ctt[ch][:, qc * 512:(qc + 1) * 512],
                            start=(ch == 0), stop=(ch == 1))
                ot = s6.tile([128, NQ], F32, tag="ot")
                nc.vector.scalar_tensor_tensor(
                    out=ot, in0=ps[:, 0:NQ],
                    scalar=bout_sb[:, oh:oh + 1], in1=qTf[oh][:, 0:NQ],
                    op0=A.add, op1=A.add)
                nc.sync.dma_start(out=outT[oh], in_=ot)
            if debug:
                for ch in range(2):
                    ctf = s6.tile([128, QT], F32, tag="ctf")
                    nc.vector.tensor_copy(ctf, ctt[ch])
                    nc.sync.dma_start(out=dbg_ct[ch], in_=ctf)

    nc.finalize()
    return nc


def _prep_shared(inputs):
    bf = ml_dtypes.bfloat16
    W_val = np.asarray(inputs["W_val"], np.float32)
    W_off = np.asarray(inputs["W_off"], np.float32)
    W_attn = np.asarray(inputs["W_attn"], np.float32)
    W_out = np.asarray(inputs["W_out"], np.float32)
    b_off = np.asarray(inputs["b_off"], np.float32)
    b_attn = np.asarray(inputs["b_attn"], np.float32)
    b_val = np.asarray(inputs["b_val"], np.float32)
    b_out = np.asarray(inputs["b_out"], np.float32)
    assert np.allclose(b_val, 0.0), "kernel assumes b_val == 0"
    woa = np.concatenate([W_off, W_attn], axis=1)
    boa = np.concatenate([b_off, b_attn], axis=0)
    e2s = np.zeros((128, 64, 128), np.float32)
    for hp in range(32):
        for sx in range(2):
            r = hp * 2 + sx
            e2s[hp * 4 + 2 * sx, r, 0:32] = 1.0
            e2s[hp * 4 + 2 * sx + 1, r, 32:64] = 1.0
    e2s = e2s.reshape(128, 64 * 128)
    r2 = np.zeros((128, 32), np.float32)
    r2[np.arange(32), np.arange(32)] = 1.0
    r2[32 + np.arange(32), np.arange(32)] = 1.0
    idf = np.eye(128, dtype=np.float32)
    fp8 = ml_dtypes.float8_e4m3
    wv8 = (W_val.reshape(2, 128, 256).transpose(1, 0, 2) * WV_SCALE)
    return dict(
        wv=np.ascontiguousarray(wv8).astype(fp8),
        woa=np.ascontiguousarray(woa.reshape(2, 128, 96)).astype(bf),
        boa=boa,
        wout=np.ascontiguousarray(W_out.reshape(2, 128, 256)).astype(bf),
        bout=b_out,
        e2s=e2s.astype(bf), r2=r2.astype(bf), idf=idf, idb=idf.astype(bf))


def make_in_maps(inputs):
    shared = _prep_shared(inputs)
    q = np.asarray(inputs["query"], np.float32)
    v = np.asarray(inputs["value"], np.float32)
    ref = np.asarray(inputs["reference_points"], np.float32)
    in_maps = []
    for c in range(N_CORES):
        in_maps.append(dict(
            q=np.ascontiguousarray(q[:, c, :]),
            v=np.ascontiguousarray(v[:, c, :]),
            ref=np.ascontiguousarray(ref[c, :, 0, :]),
            **shared))
    return in_maps


def kernel(**inputs):
    if "nc" not in _CACHE:
        _CACHE["nc"] = build_kernel(debug=False)
    nc = _CACHE["nc"]
    in_maps = make_in_maps(inputs)
    res = run_bass_kernel_spmd(nc, in_maps, core_ids=list(range(N_CORES)))
    out = np.empty((NQ, N_CORES, EMB), np.float32)
    for c in range(N_CORES):
        oT = res.results[c]["outT"]
        out[:, c, :] = oT.reshape(256, NQ).T
    return out



# revision 29
# speedup vs baseline: 1.0709x; 1.0709x over previous
import sys
sys.path.insert(0, "/opt/trn_rl_repo")

import numpy as np
import ml_dtypes
from contextlib import ExitStack

import concourse.bass as bass
import concourse.mybir as mybir
import concourse.tile as tile
from concourse import bacc
from concourse.bass_utils import run_bass_kernel_spmd

# ---- problem constants (hardcoded per spec) ----
H, W = 200, 100
NQ, NV, EMB, HEADS, NPT, DH = 2000, 20000, 256, 8, 4, 32
N_CORES = 8
YP = 203                    # padded rows y' = y+1, y' in [0, 202]
NCELL = YP * W              # 20300
NCELL_PAD = 20352           # 159 * 128
NTILE_C = NCELL_PAD // 128  # 159
VEXT = 20452                # 100 zero cols + 20000 real + 352 zero
VEXTA = 20480               # vt allocation stride (ldweights needs alignment)
QT = 2048                   # padded queries (16 tiles of 128)
NQT = 16
F32 = mybir.dt.float32
BF16 = mybir.dt.bfloat16
FP8 = mybir.dt.float8e4
I16 = mybir.dt.int16
WV_SCALE = 32.0             # W_val prescale for fp8; undone via softmax recip

_CACHE = {}


def build_kernel(debug=False):
    nc = bacc.Bacc("TRN2", target_bir_lowering=False, debug=False,
                   num_devices=N_CORES, num_swdge_queues=4,
                   dynamic_dma_scratch_size=32768)
    A = mybir.AluOpType
    ACT = mybir.ActivationFunctionType
    dt = nc.dram_tensor
    q_in = dt("q", [NQ, EMB], F32, kind="ExternalInput")
    v_in = dt("v", [NV, EMB], F32, kind="ExternalInput")
    ref_in = dt("ref", [NQ, 2], F32, kind="ExternalInput")
    wv_in = dt("wv", [128, 2, EMB], FP8, kind="ExternalInput")
    woa_in = dt("woa", [2, 128, 96], BF16, kind="ExternalInput")
    boa_in = dt("boa", [96], F32, kind="ExternalInput")
    wout_in = dt("wout", [2, 128, 256], BF16, kind="ExternalInput")
    bout_in = dt("bout", [256], F32, kind="ExternalInput")
    e2s_in = dt("e2s", [128, 64 * 128], BF16, kind="ExternalInput")
    r2_in = dt("r2", [128, 32], BF16, kind="ExternalInput")
    idf_in = dt("idf", [128, 128], F32, kind="ExternalInput")
    idb_in = dt("idb", [128, 128], BF16, kind="ExternalInput")
    outT = dt("outT", [2, 128, NQ], F32, kind="ExternalOutput")
    if debug:
        dbg_oa = dt("dbg_oa", [128, NQT * 96], F32, kind="ExternalOutput")
        dbg_wc = dt("dbg_wc", [128, NQT * 128], F32, kind="ExternalOutput")
        dbg_idx = dt("dbg_idx", [64, QT], I16, kind="ExternalOutput")
        dbg_ct = dt("dbg_ct", [2, 128, QT], F32, kind="ExternalOutput")
        dbg_vp2 = dt("dbg_vp2", [NCELL_PAD * HEADS * 64], BF16,
                     kind="ExternalOutput")

    with tile.TileContext(nc) as tc, ExitStack() as octx:
        const = octx.enter_context(tc.tile_pool(name="const", bufs=1))
        dram = octx.enter_context(tc.tile_pool(name="dram", bufs=1,
                                               space="DRAM"))
        vp2t = dram.tile([NCELL_PAD * HEADS * 64], BF16, tag="vp2")
        idxd = dram.tile([64 * QT], I16, tag="idxd")

        idf = const.tile([128, 128], F32, tag="idf")
        nc.sync.dma_start(out=idf, in_=idf_in[:, :])
        idb = const.tile([128, 128], BF16, tag="idb")
        nc.sync.dma_start(out=idb, in_=idb_in[:, :])
        wv8 = const.tile([128, 2, EMB], FP8, tag="wv8")
        nc.sync.dma_start(out=wv8, in_=wv_in[:, :, :])
        woa = [const.tile([128, 96], BF16, tag=f"woa{ch}", name=f"woa{ch}") for ch in range(2)]
        wout = [const.tile([128, 256], BF16, tag=f"wo{ch}", name=f"wo{ch}") for ch in range(2)]
        for ch in range(2):
            nc.sync.dma_start(out=woa[ch], in_=woa_in[ch])
            nc.sync.dma_start(out=wout[ch], in_=wout_in[ch])
        bias_oa = const.tile([128, 96], F32, tag="boa")
        nc.gpsimd.dma_start(out=bias_oa, in_=bass.AP(
            tensor=boa_in, offset=0, ap=[[0, 128], [1, 96]]))
        bout_sb = const.tile([128, 2], F32, tag="bout")
        nc.gpsimd.dma_start(out=bout_sb, in_=bass.AP(
            tensor=bout_in, offset=0, ap=[[1, 128], [128, 2]]))

        persist = octx.enter_context(tc.tile_pool(name="persist", bufs=1))
        qTf = [persist.tile([128, QT], F32, tag=f"qTf{c}", name=f"qTf{c}") for c in range(2)]
        qTb = [persist.tile([128, QT], BF16, tag=f"qTb{c}", name=f"qTb{c}") for c in range(2)]
        oa_sb = persist.tile([128, NQT, 96], F32, tag="oa")
        wcb = persist.tile([128, NQT, 32, 4], BF16, tag="wcb")
        ctt = [persist.tile([128, QT], BF16, tag=f"ct{c}", name=f"ct{c}") for c in range(2)]

        # ---------- stage 0: transpose query -> qT ----------
        with tc.tile_pool(name="s0", bufs=3) as s0, \
             tc.tile_pool(name="s0p", bufs=2, space="PSUM") as s0p:
            for ch in range(2):
                for g in range(4):
                    ps = s0p.tile([128, 512], F32, tag="qt_ps")
                    for k in range(4):
                        t = g * 4 + k
                        qt = s0.tile([128, 256], F32, tag="q_ld")
                        if t == NQT - 1:
                            nc.vector.memset(qt, 0.0)
                            nc.scalar.dma_start(out=qt[:NQ - 1920, :],
                                                in_=q_in[1920:NQ, :])
                        else:
                            nc.scalar.dma_start(
                                out=qt, in_=q_in[t * 128:(t + 1) * 128, :])
                        nc.tensor.transpose(ps[:, k * 128:(k + 1) * 128],
                                            qt[:, ch * 128:(ch + 1) * 128],
                                            idf)
                    nc.scalar.activation(qTf[ch][:, g * 512:(g + 1) * 512],
                                         ps, ACT.Copy)
            for ch in range(2):
                nc.vector.tensor_copy(qTb[ch], qTf[ch])

        # ---------- stage 3: off/attn projections ----------
        with tc.tile_pool(name="s3p", bufs=2, space="PSUM") as s3p:
            for t in range(NQT):
                ps = s3p.tile([128, 96], F32, tag="oa_ps")
                for ch in range(2):
                    nc.tensor.matmul(ps, qTb[ch][:, t * 128:(t + 1) * 128],
                                     woa[ch], start=(ch == 0), stop=(ch == 1))
                nc.vector.tensor_add(oa_sb[:, t, :], ps, bias_oa)
            if debug:
                nc.sync.dma_start(out=dbg_oa[:, :],
                                  in_=oa_sb.rearrange("p t c -> p (t c)"))

        # ---------- stage 4: coords, weights, indices ----------
        with tc.tile_pool(name="s4", bufs=1) as s4, \
             tc.tile_pool(name="s4p", bufs=2, space="PSUM") as s4p:
            shp = [128, NQT, 32]
            ref_sb = s4.tile([128, NQT, 2], F32, tag="ref")
            nc.vector.memset(ref_sb, 0.0)
            nc.scalar.dma_start(
                out=ref_sb[:, 0:15, :],
                in_=ref_in[0:1920, :].rearrange("(t p) c -> p t c", p=128))
            nc.scalar.dma_start(out=ref_sb[:NQ - 1920, 15, :],
                                in_=ref_in[1920:NQ, :])
            oav = oa_sb.rearrange("p t (c two) -> p t c two", two=2)
            ox = oav[:, :, 0:32, 0]
            oy = oav[:, :, 0:32, 1]
            awl = oa_sb[:, :, 64:96]

            awe = s4.tile(shp, F32, tag="awe")
            nc.scalar.activation(awe, awl, ACT.Exp)
            s1t = s4.tile([128, NQT, 16], F32, tag="s1t")
            av = awe.rearrange("p t (c two) -> p t c two", two=2)
            nc.vector.tensor_add(s1t, av[:, :, :, 0], av[:, :, :, 1])
            s2t = s4.tile([128, NQT, 8], F32, tag="s2t")
            sv = s1t.rearrange("p t (c two) -> p t c two", two=2)
            nc.vector.tensor_add(s2t, sv[:, :, :, 0], sv[:, :, :, 1])
            # fold the fp8 W_val prescale out of the softmax normalization
            nc.vector.tensor_scalar(out=s2t, in0=s2t, scalar1=WV_SCALE,
                                    scalar2=None, op0=A.mult)
            rec = s4.tile([128, NQT, 8], F32, tag="rec")
            nc.vector.reciprocal(rec, s2t)
            awn = s4.tile(shp, F32, tag="awn")
            nc.vector.tensor_mul(
                awn.rearrange("p t (c f) -> p t c f", f=4),
                awe.rearrange("p t (c f) -> p t c f", f=4),
                rec[:, :, :, None].broadcast_to([128, NQT, 8, 4]))

            refw = s4.tile([128, NQT, 2], F32, tag="refw")
            nc.vector.tensor_scalar(out=refw[:, :, 0:1],
                                    in0=ref_sb[:, :, 0:1],
                                    scalar1=float(W), scalar2=0.5,
                                    op0=A.mult, op1=A.add)
            nc.vector.tensor_scalar(out=refw[:, :, 1:2],
                                    in0=ref_sb[:, :, 1:2],
                                    scalar1=float(H), scalar2=0.5,
                                    op0=A.mult, op1=A.add)
            px = s4.tile(shp, F32, tag="px")
            nc.vector.tensor_add(px, ox, refw[:, :, 0:1].broadcast_to(shp))
            py = s4.tile(shp, F32, tag="py")
            nc.vector.tensor_add(py, oy, refw[:, :, 1:2].broadcast_to(shp))
            nc.vector.tensor_scalar(out=px, in0=px, scalar1=0.0,
                                    scalar2=float(W + 1),
                                    op0=A.max, op1=A.min)
            nc.vector.tensor_scalar(out=py, in0=py, scalar1=0.0,
                                    scalar2=float(YP - 2),
                                    op0=A.max, op1=A.min)
            M23 = float(1 << 23)
            x0 = s4.tile(shp, F32, tag="x0")
            nc.vector.tensor_scalar(out=x0, in0=px, scalar1=M23 - 0.5,
                                    scalar2=M23, op0=A.add, op1=A.subtract)
            y0 = s4.tile(shp, F32, tag="y0")
            nc.vector.tensor_scalar(out=y0, in0=py, scalar1=M23 - 0.5,
                                    scalar2=M23, op0=A.add, op1=A.subtract)
            fx = s4.tile(shp, F32, tag="fx")
            nc.vector.tensor_sub(fx, px, x0)
            fy = s4.tile(shp, F32, tag="fy")
            nc.vector.tensor_sub(fy, py, y0)

            idxf = s4.tile([128, NQT, 64], F32, tag="idxf")
            idv = idxf.rearrange("p t (c two) -> p t c two", two=2)
            cellf = s4.tile(shp, F32, tag="cellf")
            nc.vector.scalar_tensor_tensor(out=cellf, in0=y0,
                                           scalar=float(W), in1=x0,
                                           op0=A.mult, op1=A.add)
            nc.vector.tensor_scalar(out=idv[:, :, :, 0], in0=cellf,
                                    scalar1=1.0, scalar2=0.0,
                                    op0=A.subtract, op1=A.max)
            nc.vector.tensor_scalar(out=idv[:, :, :, 0], in0=idv[:, :, :, 0],
                                    scalar1=float(NCELL - 1), scalar2=None,
                                    op0=A.min)
            nc.vector.tensor_scalar(out=idv[:, :, :, 1], in0=idv[:, :, :, 0],
                                    scalar1=1.0, scalar2=float(NCELL),
                                    op0=A.add, op1=A.min)

            ga1 = s4.tile(shp, F32, tag="ga1")
            nc.vector.tensor_scalar(out=ga1, in0=x0, scalar1=0.5,
                                    scalar2=None, op0=A.is_ge)
            ga2 = s4.tile(shp, F32, tag="ga2")
            nc.vector.tensor_scalar(out=ga2, in0=x0, scalar1=float(W) + 0.5,
                                    scalar2=None, op0=A.is_le)
            gb = s4.tile(shp, F32, tag="gb")
            nc.vector.tensor_scalar(out=gb, in0=x0, scalar1=float(W) - 0.5,
                                    scalar2=None, op0=A.is_le)
            fx1 = s4.tile(shp, F32, tag="fx1")
            nc.vector.tensor_scalar(out=fx1, in0=fx, scalar1=-1.0,
                                    scalar2=1.0, op0=A.mult, op1=A.add)
            fy1 = s4.tile(shp, F32, tag="fy1")
            nc.vector.tensor_scalar(out=fy1, in0=fy, scalar1=-1.0,
                                    scalar2=1.0, op0=A.mult, op1=A.add)
            aa = s4.tile(shp, F32, tag="aa")
            nc.vector.tensor_mul(aa, fx1, ga1)
            nc.vector.tensor_mul(aa, aa, ga2)
            nc.vector.tensor_mul(aa, aa, awn)
            bb = s4.tile(shp, F32, tag="bb")
            nc.vector.tensor_mul(bb, fx, gb)
            nc.vector.tensor_mul(bb, bb, awn)

            wc = s4.tile([128, NQT, 128], F32, tag="wc")
            wcv = wc.rearrange("p t (c j) -> p t c j", j=4)
            nc.vector.tensor_mul(wcv[:, :, :, 0], aa, fy1)
            nc.vector.tensor_mul(wcv[:, :, :, 1], aa, fy)
            nc.vector.tensor_mul(wcv[:, :, :, 2], bb, fy1)
            nc.vector.tensor_mul(wcv[:, :, :, 3], bb, fy)
            if debug:
                nc.sync.dma_start(out=dbg_wc[:, :],
                                  in_=wc.rearrange("p t c -> p (t c)"))

            nc.scalar.activation(
                wcb, wc.rearrange("p t (c j) -> p t c j", j=4), ACT.Copy)
            idxT = s4.tile([64, QT], F32, tag="idxT")
            for t in range(NQT):
                ps2 = s4p.tile([128, 128], F32, tag="tr2_ps")
                nc.tensor.transpose(ps2[:64, :], idxf[:, t, :], idf)
                dstv = idxT.rearrange("p (v tt u) -> p v tt u",
                                      v=16, tt=16)[:, :, t, :]
                srcv = ps2[:64, :].rearrange("p (u v) -> p v u", u=8, v=16)
                nc.vector.tensor_copy(dstv, srcv)
            idx16 = s4.tile([64, QT], I16, tag="idx16")
            nc.vector.tensor_copy(idx16, idxT)
            nc.scalar.dma_start(
                out=idxd[:].rearrange("(p f) -> p f", p=64), in_=idx16)
            if debug:
                nc.sync.dma_start(out=dbg_idx[:, :], in_=idx16)

        # ---------- stage 1+2: value transpose, fp8 projection, vp2 --------
        # vp2 is four head-pair tables: [pair][cell][2h][2s][32] bf16, so a
        # single 512B gather packet covers cells (c, c+1) for one head pair.
        with tc.tile_pool(name="vtp", bufs=1) as vtp:
            vt = vtp.tile([128, 2, VEXTA], FP8, tag="vt")
            for ch in range(2):
                nc.vector.memset(vt[:, ch, 0:100], 0.0)
                nc.vector.memset(vt[:, ch, 20100:VEXT], 0.0)
            with tc.tile_pool(name="s1f", bufs=5) as s1f, \
                 tc.tile_pool(name="s1", bufs=4) as s1, \
                 tc.tile_pool(name="s1p", bufs=2, space="PSUM") as s1p:
                NT = 157
                for g in range(40):
                    tiles = [t for t in range(g * 4, min(g * 4 + 4, NT))]
                    vbf = []
                    for t in tiles:
                        vb32 = s1f.tile([128, 256], F32, tag="v_ld32")
                        vb = s1.tile([128, 256], BF16, tag="v_ld")
                        rows = min(128, NV - t * 128)
                        if rows < 128:
                            nc.vector.memset(vb, 0.0)
                        nc.sync.dma_start(
                            out=vb32[:rows, :],
                            in_=v_in[t * 128:t * 128 + rows, :])
                        if t % 2 == 0:
                            nc.vector.tensor_copy(vb[:rows, :], vb32[:rows, :])
                        else:
                            nc.scalar.activation(vb[:rows, :], vb32[:rows, :],
                                                 ACT.Copy)
                        vbf.append(vb)
                    for ch in range(2):
                        ps = s1p.tile([128, 512], BF16, tag="vt_ps")
                        for k, vb in enumerate(vbf):
                            nc.tensor.transpose(
                                ps[:, k * 128:(k + 1) * 128],
                                vb[:, ch * 128:(ch + 1) * 128], idb)
                        cols = len(vbf) * 128
                        nc.scalar.activation(
                            vt[:, ch, 100 + g * 512:100 + g * 512 + cols],
                            ps[:, :cols], ACT.Copy)

            with tc.tile_pool(name="s2", bufs=3) as s2, \
                 tc.tile_pool(name="s2p", bufs=2, space="PSUM") as s2p:
                DR = mybir.MatmulPerfMode.DoubleRow
                for ci in range(NTILE_C):
                    ps = s2p.tile([128, 512], F32, tag="pj_ps")
                    c0 = ci * 128
                    nc.tensor.matmul(ps[:, 0:256], vt[:, :, c0:c0 + 128],
                                     wv8, start=True, stop=True,
                                     perf_mode=DR)
                    nc.tensor.matmul(ps[:, 256:512],
                                     vt[:, :, c0 + 100:c0 + 228],
                                     wv8, start=True, stop=True,
                                     perf_mode=DR)
                    mx = s2.tile([128, 512], BF16, tag="mx")
                    src = ps.rearrange("p (s h d) -> p h s d", s=2, h=8)
                    dst = mx.rearrange("p (h s d) -> p h s d", h=8, s=2)
                    if ci % 2 == 0:
                        nc.vector.tensor_copy(dst, src)
                    else:
                        nc.scalar.activation(dst, src, ACT.Copy)
                    nc.sync.dma_start(
                        out=bass.AP(
                            tensor=vp2t[:].tensor,
                            offset=vp2t[:].offset + ci * 16384,
                            ap=[[128, 128], [NCELL_PAD * 128, 4], [1, 128]]),
                        in_=mx.rearrange("p (pr e) -> p pr e", pr=4))
        if debug:
            nc.sync.dma_start(out=dbg_vp2[:], in_=vp2t[:])

        # ---------- stage 5: gather (q-major) + weighted combine on DVE ----
        # Non-transpose gathers are multi-queue safe (no xbar state); output
        # lands query-major: g[p, t, e] = packet for query q = t*128+p.
        vp2full = vp2t[:]
        with tc.tile_pool(name="s5i", bufs=1) as s5i, \
             tc.tile_pool(name="s5", bufs=10) as s5, \
             tc.tile_pool(name="s5b", bufs=4) as s5b, \
             tc.tile_pool(name="s5a", bufs=2) as s5a:
            # all 64 wrapped idx rows in one load (keeps GPSIMD free of
            # small DMAs): iball[p, row, c] = idxd[row*QT + (p%16)*128 + c]
            iball = s5i.tile([128, 64, 128], I16, tag="iball")
            for rep in range(8):
                nc.scalar.dma_start(
                    out=iball[16 * rep:16 * rep + 16, :, :],
                    in_=bass.AP(
                        tensor=idxd[:].tensor, offset=idxd[:].offset,
                        ap=[[128, 16], [QT, 64], [1, 128]]))
            ctq = s5i.tile([128, NQT, HEADS, 32], BF16, tag="ctq")
            for h in range(HEADS):
                acc = s5a.tile([128, NQT, 32], F32, tag="acc")
                for p in range(NPT):
                    hp = h * 4 + p
                    g = s5.tile([128, NQT, 256], BF16, tag="g")
                    nc.gpsimd.dma_gather(
                        g,
                        bass.AP(tensor=vp2full.tensor,
                                offset=(vp2full.offset
                                        + (h // 2) * NCELL_PAD * 128
                                        + (h % 2) * 64),
                                ap=[[128, NCELL_PAD - 2], [1, 256]]),
                        iball[:, hp * 2, :], QT, QT, 256, elem_step=128,
                        transpose=False, single_packet=False,
                        queue_num=hp % 4)
                    # packet = [cell c: pair(2h x 2s x 32) | cell c+1: ...];
                    # useful quarters: x (cell) in {0,1}, head-slot h%2,
                    # rows r in {0,1} -> corner weight j = x*2 + r
                    gv = g.rearrange("p t (x q d) -> p t x q d",
                                     x=2, q=4)[:, :, :, 0:2, :]
                    m = s5b.tile([128, NQT, 2, 2, 32], BF16, tag="m")
                    w5 = wcb[:, :, hp, :].rearrange(
                        "p t (x r) -> p t x r", x=2)
                    nc.vector.tensor_mul(
                        m, gv,
                        w5[:, :, :, :, None].broadcast_to(
                            [128, NQT, 2, 2, 32]))
                    t2 = s5b.tile([128, NQT, 2, 32], BF16, tag="t2")
                    nc.vector.tensor_add(t2, m[:, :, :, 0, :],
                                         m[:, :, :, 1, :])
                    if p == 0:
                        nc.vector.tensor_add(acc, t2[:, :, 0, :],
                                             t2[:, :, 1, :])
                    else:
                        tmp = s5b.tile([128, NQT, 32], BF16, tag="tmp")
                        nc.vector.tensor_add(tmp, t2[:, :, 0, :],
                                             t2[:, :, 1, :])
                        nc.vector.tensor_add(acc, acc, tmp)
                nc.scalar.activation(ctq[:, :, h, :], acc, ACT.Copy)
                if h % 4 == 3:
                    # heads (h-3..h) complete -> transpose this emb half now
                    # so ch-0's transposes hide under heads 4-7's gathers
                    ch = h // 4
                    with tc.tile_pool(name=f"s5p{ch}", bufs=2,
                                      space="PSUM") as s5p:
                        for tg in range(4):
                            ps = s5p.tile([128, 512], BF16, tag="ct_ps")
                            for k in range(4):
                                t = tg * 4 + k
                                src = ctq.rearrange("p t h d -> p t (h d)")[
                                    :, t, ch * 128:(ch + 1) * 128]
                                nc.tensor.transpose(
                                    ps[:, k * 128:(k + 1) * 128], src, idb)
                            dst = ctt[ch][:, tg * 512:(tg + 1) * 512]
                            if tg % 2 == 0:
                                nc.vector.tensor_copy(dst, ps)
                            else:
                                nc.scalar.activation(dst, ps, ACT.Copy)

        # ---------- stage 6: output projection + identity ----------
        with tc.tile_pool(name="s6", bufs=2) as s6, \
             tc.tile_pool(name="s6p", bufs=2, space="PSUM") as s6p:
            for oh in range(2):
                ps = s6p.tile([128, QT], F32, tag="out_ps")
                for qc in range(4):
                    for ch in range(2):
                        nc.tensor.matmul(
                            ps[:, qc * 512:(qc + 1) * 512],
                            wout[ch][:, oh * 128:(oh + 1) * 128],
                            ctt[ch][:, qc * 512:(qc + 1) * 512],
                            start=(ch == 0), stop=(ch == 1))
                ot = s6.tile([128, NQ], F32, tag="ot")
                nc.vector.scalar_tensor_tensor(
                    out=ot, in0=ps[:, 0:NQ],
                    scalar=bout_sb[:, oh:oh + 1], in1=qTf[oh][:, 0:NQ],
                    op0=A.add, op1=A.add)
                nc.sync.dma_start(out=outT[oh], in_=ot)
            if debug:
                for ch in range(2):
                    ctf = s6.tile([128, QT], F32, tag="ctf")
                    nc.vector.tensor_copy(ctf, ctt[ch])
                    nc.sync.dma_start(out=dbg_ct[ch], in_=ctf)

    nc.finalize()
    return nc


def _prep_shared(inputs):
    bf = ml_dtypes.bfloat16
    W_val = np.asarray(inputs["W_val"], np.float32)
    W_off = np.asarray(inputs["W_off"], np.float32)
    W_attn = np.asarray(inputs["W_attn"], np.float32)
    W_out = np.asarray(inputs["W_out"], np.float32)
    b_off = np.asarray(inputs["b_off"], np.float32)
    b_attn = np.asarray(inputs["b_attn"], np.float32)
    b_val = np.asarray(inputs["b_val"], np.float32)
    b_out = np.asarray(inputs["b_out"], np.float32)
    assert np.allclose(b_val, 0.0), "kernel assumes b_val == 0"
    woa = np.concatenate([W_off, W_attn], axis=1)
    boa = np.concatenate([b_off, b_attn], axis=0)
    e2s = np.zeros((128, 64, 128), np.float32)
    for hp in range(32):
        for sx in range(2):
            r = hp * 2 + sx
            e2s[hp * 4 + 2 * sx, r, 0:32] = 1.0
            e2s[hp * 4 + 2 * sx + 1, r, 32:64] = 1.0
    e2s = e2s.reshape(128, 64 * 128)
    r2 = np.zeros((128, 32), np.float32)
    r2[np.arange(32), np.arange(32)] = 1.0
    r2[32 + np.arange(32), np.arange(32)] = 1.0
    idf = np.eye(128, dtype=np.float32)
    fp8 = ml_dtypes.float8_e4m3
    wv8 = (W_val.reshape(2, 128, 256).transpose(1, 0, 2) * WV_SCALE)
    return dict(
        wv=np.ascontiguousarray(wv8).astype(fp8),
        woa=np.ascontiguousarray(woa.reshape(2, 128, 96)).astype(bf),
        boa=boa,
        wout=np.ascontiguousarray(W_out.reshape(2, 128, 256)).astype(bf),
        bout=b_out,
        e2s=e2s.astype(bf), r2=r2.astype(bf), idf=idf, idb=idf.astype(bf))


def make_in_maps(inputs):
    shared = _prep_shared(inputs)
    q = np.asarray(inputs["query"], np.float32)
    v = np.asarray(inputs["value"], np.float32)
    ref = np.asarray(inputs["reference_points"], np.float32)
    in_maps = []
    for c in range(N_CORES):
        in_maps.append(dict(
            q=np.ascontiguousarray(q[:, c, :]),
            v=np.ascontiguousarray(v[:, c, :]),
            ref=np.ascontiguousarray(ref[c, :, 0, :]),
            **shared))
    return in_maps


def kernel(**inputs):
    if "nc" not in _CACHE:
        _CACHE["nc"] = build_kernel(debug=False)
    nc = _CACHE["nc"]
    in_maps = make_in_maps(inputs)
    res = run_bass_kernel_spmd(nc, in_maps, core_ids=list(range(N_CORES)))
    out = np.empty((NQ, N_CORES, EMB), np.float32)
    for c in range(N_CORES):
        oT = res.results[c]["outT"]
        out[:, c, :] = oT.reshape(256, NQ).T
    return out

